# revision 18
# baseline (speedup 1.0000x reference)
"""Bass/Trainium2 kernel for causal multi-head attention.

B=2, S=2048, DIM=2048, H=16 heads, HD=128.
Sharding: 8 cores = (batch b in 0..1) x (head-group g in 0..3).
Each core column-shards wq/wk/wv (4 heads = 512 cols), row-shards wo,
and produces a partial [S, DIM] output; the host sums the 4 partials
per batch (unshard of the row-sharded wo matmul).

Device-side layout trick: the host feeds q/k/v pre-transposed (qT =
q[b].T etc.), so every matmul contraction lands on the partition dim
with zero on-device transposes:
  xqT[hd, s] = wq_chunk.T @ qT_chunk          (proj, transposed out)
  xkT[hd, s] likewise; xv[s, hd] from vT as lhsT
  ST[k, q]   = xkT_chunk.T @ xqT_slice        (scores, transposed)
  softmax over k = partition dim: sum via ones-column matmul
  attnT[hd, q] = xv_chunk.T @ probsT          (PV; output pre-transposed)
  out[s, dout] = attnT_chunk.T @ wo_chunk     (final projection)

Schedule: projections q,k then v st-by-st with attention q-tiles 0 and 1
merged into the later v stages (scope 1); scope 2 runs q-tiles 2..3 with
a 2-deep score->exp->pv software pipeline interleaved with the output
projection, so the PE never waits on the Scalar engine's exp.
"""

import sys

sys.path.insert(0, "/opt/trn_rl_repo")

import numpy as np

B, S, DIM, H = 2, 2048, 2048, 16
HD = 128
NCORES = 8
GROUPS = 4  # head-groups (tensor parallel)
HPG = H // GROUPS  # 4 heads per group
DG = HPG * HD  # 512 = per-group projection width
P = 128
DC = DIM // P  # 16 din chunks
ST_N = S // 512  # 4 s-tiles of 512
SC_N = S // P  # 16 s-chunks of 128
SCALE = 1.0 / np.sqrt(HD)

_cache = {}


def _build(reps=1):
    from contextlib import ExitStack

    import concourse.bacc as bacc
    import concourse.mybir as mybir
    import concourse.tile as tile

    f32 = mybir.dt.float32
    f16 = mybir.dt.float16
    Exp = mybir.ActivationFunctionType.Exp

    nc = bacc.Bacc("TRN2", target_bir_lowering=False, debug=False,
                   num_devices=NCORES)

    # host-tiled layouts: every DMA reads a fully contiguous block
    qT = nc.dram_tensor("qT", [DC, ST_N, P, 512], f16, kind="ExternalInput")
    kT = nc.dram_tensor("kT", [DC, ST_N, P, 512], f16, kind="ExternalInput")
    vT = nc.dram_tensor("vT", [DC, ST_N, P, 512], f16, kind="ExternalInput")
    wq = nc.dram_tensor("wq", [4, 4, P, DG], f16, kind="ExternalInput")
    wk = nc.dram_tensor("wk", [4, 4, P, DG], f16, kind="ExternalInput")
    wv = nc.dram_tensor("wv", [4, 4, P, DG], f16, kind="ExternalInput")
    wo = nc.dram_tensor("wo", [4, P, DIM], f16, kind="ExternalInput")
    msk = nc.dram_tensor("msk", [P, 512], f16, kind="ExternalInput")
    onesd = nc.dram_tensor("onesd", [P, P], f16, kind="ExternalInput")
    outp = nc.dram_tensor("outp", [S, DIM], f16, kind="ExternalOutput")

    with tile.TileContext(nc) as tc, ExitStack() as ctx:
        # Resident for the whole kernel: projected activations + consts.
        resid = ctx.enter_context(tc.tile_pool(name="resid", bufs=1))
        _ = reps  # body below may be repeated for timing builds
        xqT = resid.tile([P, HPG, S], f16, tag="xqT")  # [hd, head, s]
        xkT = resid.tile([P, HPG, S], f16, tag="xkT")
        xv = resid.tile([P, SC_N, DG], f16, tag="xv")  # [s%128, s//128, dout]
        ones = resid.tile([P, P], f16, tag="ones")
        attnT = resid.tile([P, HPG, S], f16, tag="attnT")
        # single lower-triangle mask: tri[p, j] = (j >= p)
        tri = resid.tile([P, 512], f16, tag="tri")
        wo_tiles = [resid.tile([P, DIM], f16, tag=f"wo{c4}",
                               name=f"wo{c4}") for c4 in range(4)]
        # consts go via the Scalar engine's DMA ring so they don't
        # delay the weight/activation loads gating the first matmul
        nc.scalar.dma_start(ones[:], onesd[:])
        nc.scalar.dma_start(tri[:], msk[:])
        # warm the Exp activation table during the projections so the
        # first attention exp doesn't pay the 1.3us ACT_TABLE_LOAD
        actwarm = resid.tile([1, 8], mybir.dt.float32, tag="actwarm")
        nc.vector.memzero(actwarm[:])
        nc.scalar.activation(actwarm[:], actwarm[:],
                             mybir.ActivationFunctionType.Exp)
        # dummy matmuls on a zeroed tile keep the PE busy while the
        # first weight/activation DMAs land, so the p-state governor
        # has the array at full clock when the real work starts
        warm = resid.tile([P, 512], f16, tag="warm")
        nc.vector.memzero(warm[:])

        def merge(*streams):
            """Proportional round-robin merge of (units, weight) streams."""
            streams = [(list(u), w) for u, w in streams if u]
            pos = [0] * len(streams)
            out = []
            while True:
                best, bf = -1, None
                for si, (u, w) in enumerate(streams):
                    if pos[si] >= len(u):
                        continue
                    f = pos[si] / len(u)
                    if bf is None or f < bf:
                        best, bf = si, f
                if best < 0:
                    return out
                out.append(streams[best][0][pos[best]])
                pos[best] += 1
            return out

        def run(units):
            for u in units:
                u()

        for _rep in range(reps):
            if True:
                # ---------- projection unit builders ----------
                def proj_units(name, w_tiles, in_dram, streampool, ppsum):
                    """One tensor's projection as a list of units."""
                    units = []
                    state = {}

                    def w_slice(d, csl):
                        return w_tiles[d // 4][:, d % 4, csl]

                    for st in range(ST_N):
                        def u_alloc(st=st):
                            state[st] = [
                                ppsum.tile([P, 512], f32, tag="pp",
                                           name=f"pp{name}{st}{_i}")
                                for _i in range(4)]
                        for eighth in range(8):
                            def u(st=st, eighth=eighth, alloc=(eighth == 0)):
                                if alloc:
                                    u_alloc(st)
                                psums = state[st]
                                x_sb = streampool.tile([P, 2, 512], f16,
                                                       tag="xs", name="x_sb")
                                src = in_dram[2 * eighth:2 * eighth + 2, st]
                                nc.sync.dma_start(
                                    x_sb[:], src.rearrange("d p j -> p d j"))
                                for i in range(2):
                                    d = 2 * eighth + i
                                    if name == "v":
                                        for j in range(4):
                                            nc.tensor.matmul(
                                                psums[j][:],
                                                x_sb[:, i, j * P:(j + 1) * P],
                                                w_slice(d, slice(None)),
                                                start=(d == 0),
                                                stop=(d == DC - 1))
                                    else:
                                        for c in range(4):
                                            nc.tensor.matmul(
                                                psums[c][:],
                                                w_slice(d,
                                                        slice(c * P,
                                                              (c + 1) * P)),
                                                x_sb[:, i, :],
                                                start=(d == 0),
                                                stop=(d == DC - 1))
                                if eighth == 7:
                                    for i in range(4):
                                        if name == "q":
                                            nc.vector.tensor_copy(
                                                xqT[:, i,
                                                    st * 512:(st + 1) * 512],
                                                psums[i][:])
                                        elif name == "k":
                                            nc.vector.tensor_copy(
                                                xkT[:, i,
                                                    st * 512:(st + 1) * 512],
                                                psums[i][:])
                                        else:
                                            nc.vector.tensor_copy(
                                                xv[:, 4 * st + i, :],
                                                psums[i][:])
                            units.append(u)
                    return units

                def alloc_w(pool, name, w_dram, split_first=False):
                    """Weight tiles + deferred per-chunk DMA closures.
                    split_first halves the first chunk's DMA so the
                    kernel's first matmul (needing only d=0,1) starts
                    ~3us sooner."""
                    tiles = [None] * 4
                    dmas = []
                    for wq4 in range(4):
                        def dma(wq4=wq4):
                            wt = pool.tile([P, 4, DG], f16, tag="w",
                                           name=f"w{name}{wq4}")
                            if wq4 == 0 and split_first:
                                for h in range(2):
                                    nc.sync.dma_start(
                                        wt[:, 2 * h:2 * h + 2],
                                        w_dram[wq4, 2 * h:2 * h + 2]
                                        .rearrange("d p n -> p d n"))
                            else:
                                nc.sync.dma_start(
                                    wt[:],
                                    w_dram[wq4].rearrange("d p n -> p d n"))
                            tiles[wq4] = wt
                        dmas.append(dma)
                    return tiles, dmas

                def weave(units, dmas, slots):
                    """Insert dma closures before the given unit indices."""
                    out = []
                    for i, u in enumerate(units):
                        while dmas and slots and i == slots[0]:
                            out.append(dmas.pop(0))
                            slots.pop(0)
                        out.append(u)
                    return out

                # ---------- attention unit builders ----------
                def a_units2(qt, c, apool, aux, stp, pvp, sump):
                    """3-deep pipelined attention chain (scope 2): the
                    sum/pv matmuls consume probsT from two chunks back,
                    so exp+mask latency never stalls the PE. The softmax
                    denominator matmul uses a full [128,128] ones
                    stationary: same row cost as M=1, but the PE tile
                    config stays 128x128 for every matmul in the kernel
                    (no reconfig stalls) and the denominator arrives
                    pre-broadcast across partitions, so the normalize is
                    a plain DVE mul (no broadcast matmul)."""
                    units = []
                    nkc = 4 * qt + 4
                    kc_order = list(range(4 * qt, nkc)) + list(range(4 * qt))
                    state = {}

                    def geom(ui):
                        kc = kc_order[ui]
                        d = kc - 4 * qt
                        off = max(0, d) * P
                        return kc, d, off, 512 - off

                    for ui in range(nkc + 2):
                        def u(qt=qt, c=c, ui=ui, nkc=nkc):
                            if ui < nkc:
                                kc, d, off, w = geom(ui)
                                qsl = slice(qt * 512 + off, (qt + 1) * 512)
                                st_ps = stp.tile([P, 512], f32, tag="st",
                                                 name="st2_ps")
                                nc.tensor.matmul(
                                    st_ps[:, :w],
                                    xkT[:, c, kc * P:(kc + 1) * P],
                                    xqT[:, c, qsl],
                                    start=True, stop=True)
                                probsT = apool.tile([P, 512], f16,
                                                    tag="probsT",
                                                    name="probsT2")
                                nc.scalar.activation(
                                    probsT[:, :w], st_ps[:, :w], Exp,
                                    scale=SCALE)
                                if d >= 0:
                                    mw = min(P, w)
                                    nc.vector.tensor_mul(
                                        probsT[:, :mw], probsT[:, :mw],
                                        tri[:, :mw])
                                state[ui] = probsT
                            if ui == 0:
                                state["pv"] = pvp.tile([P, 512], f32,
                                                       tag="pv", name="pv2")
                                state["sum"] = sump.tile([P, 512], f32,
                                                         tag="sum",
                                                         name="sum2")
                            elif ui >= 2:
                                kc, d, off, w = geom(ui - 2)
                                probsT = state.pop(ui - 2)
                                nc.tensor.matmul(
                                    state["sum"][:, off:], ones[:, :],
                                    probsT[:, :w],
                                    start=(ui == 2), stop=(ui == nkc + 1))
                                nc.tensor.matmul(
                                    state["pv"][:, off:],
                                    xv[:, kc, c * P:(c + 1) * P],
                                    probsT[:, :w],
                                    start=(ui == 2), stop=(ui == nkc + 1))
                                if ui == nkc + 1:
                                    recip = aux.tile([P, 512], f32,
                                                     tag="recip",
                                                     name="recip2")
                                    nc.vector.reciprocal_approx_fast(
                                        out=recip[:], in_=state["sum"][:])
                                    nc.vector.tensor_mul(
                                        attnT[:, c,
                                              qt * 512:(qt + 1) * 512],
                                        state["pv"][:], recip[:])
                        units.append(u)
                    return units

                def o_units_for_qt(qt, opool, psum_cycle):
                    units = []
                    idx = 0
                    for sc in range(4 * qt, 4 * qt + 4):
                        for dt in range(4):
                            def u(sc=sc, dt=dt, idx=idx):
                                pool, tag = psum_cycle[idx % len(psum_cycle)]
                                o_ps = pool.tile([P, 512], f32, tag=tag,
                                                 name="o_ps")
                                for c in range(HPG):
                                    nc.tensor.matmul(
                                        o_ps[:],
                                        attnT[:, c, sc * P:(sc + 1) * P],
                                        wo_tiles[c][:,
                                                    dt * 512:(dt + 1) * 512],
                                        start=(c == 0), stop=(c == HPG - 1))
                                o_sb = opool.tile([P, 512], f16, tag="o_sb",
                                                  name="o_sb")
                                nc.vector.tensor_copy(o_sb[:], o_ps[:])
                                nc.sync.dma_start(
                                    outp[sc * P:(sc + 1) * P,
                                         dt * 512:(dt + 1) * 512],
                                    o_sb[:])
                            units.append(u)
                            idx += 1
                    return units

                # ================= schedule =================
                apool = ctx.enter_context(tc.tile_pool(name="apool", bufs=8))
                aux = ctx.enter_context(tc.tile_pool(name="aux1", bufs=2))
                # Scope 1: pure projections. The three tensors share one
                # weight pool (wv reuses wq's freed slots) and one stream
                # pool, with weight-chunk DMAs woven between units.
                with (
                    tc.tile_pool(name="wpool", bufs=8) as wpool,
                    tc.tile_pool(name="stream", bufs=6) as stream,
                    tc.tile_pool(name="ppsum", bufs=4, space="PSUM") as ppsum,
                    tc.tile_pool(name="stp0", bufs=2, space="PSUM") as stp0,
                    tc.tile_pool(name="pvp0", bufs=1, space="PSUM") as pvp0,
                    tc.tile_pool(name="sump0", bufs=1,
                                 space="PSUM") as sump0,
                ):
                    for _wi in range(40):
                        wps = ppsum.tile([P, 512], f32, tag="pp",
                                         name="warm_ps")
                        nc.tensor.matmul(wps[:], warm[:, :P], warm[:],
                                         start=True, stop=True)
                    wq_t, wq_d = alloc_w(wpool, "q", wq, split_first=True)
                    uq = proj_units("q", wq_t, qT, stream, ppsum)
                    wk_t, wk_d = alloc_w(wpool, "k", wk)
                    uk = proj_units("k", wk_t, kT, stream, ppsum)
                    wv_t, wv_d = alloc_w(wpool, "v", wv)
                    uv = proj_units("v", wv_t, vT, stream, ppsum)
                    a0 = []
                    for c in range(HPG):
                        a0 += a_units2(0, c, apool, aux, stp0, pvp0,
                                       sump0)
                    # wq chunk i lands just before the unit that needs it;
                    # wk/wv chunks prefetch through the previous phase.
                    run(weave(uq, wq_d + wk_d, [0, 2, 4, 6, 12, 16, 20, 24]))
                    run(weave(uk, wv_d, [12, 16, 20, 24]))
                    # A(0) needs V st=0 evictions -> gate behind uv[:8]
                    run(uv[:8])
                    # prefetch wo for scope 2 while V proj streams
                    for c4 in range(4):
                        nc.sync.dma_start(wo_tiles[c4][:], wo[c4])
                    run(merge((uv[8:], 1), (a0, 1)))

                # Scope 2: attention qt 1..3 (3-deep pipelined chains) +
                # output projection one q-tile behind.
                with (
                    tc.tile_pool(name="opool", bufs=6) as opool,
                    tc.tile_pool(name="stp", bufs=2, space="PSUM") as stp,
                    tc.tile_pool(name="pvp", bufs=3, space="PSUM") as pvp,
                    tc.tile_pool(name="sump", bufs=1, space="PSUM") as sump,
                    tc.tile_pool(name="opsum", bufs=2, space="PSUM") as opsum,
                ):
                    cyc1 = [(opsum, "o")]
                    # tail-only: rotate across all four (now idle) psum tags
                    cyc4 = [(opsum, "o"), (sump, "sum"), (stp, "st"),
                            (pvp, "pv")]
                    for qt in range(1, ST_N):
                        ou = o_units_for_qt(qt - 1, opool, cyc1)
                        # cluster each chain's share of o units at its
                        # head, covering the exp pipeline-fill window
                        # (the chain is latency-self-sufficient after
                        # its first two chunks are in flight)
                        units = []
                        oi = 0
                        for c in range(HPG):
                            ch = a_units2(qt, c, apool, aux, stp, pvp,
                                          sump)
                            for i, u in enumerate(ch):
                                units.append(u)
                                if i < 4 and oi < len(ou):
                                    units.append(ou[oi])
                                    oi += 1
                        units += ou[oi:]
                        run(units)
                    run(o_units_for_qt(3, opool, cyc4))
    nc.compile()
    return nc


def _get_nc(reps=1):
    key = ("nc", reps)
    if key not in _cache:
        _cache[key] = _build(reps)
    return _cache[key]


def _host_inputs(q, k, v, wq, wk, wv, wo):
    pp = np.arange(P)[:, None]
    jj = np.arange(512)[None, :]
    mask = np.ascontiguousarray((jj >= pp).astype(np.float16))
    ones = np.ones((P, P), np.float16)
    in_maps = []
    for core in range(NCORES):
        b, g = divmod(core, GROUPS)
        sl = slice(g * DG, (g + 1) * DG)
        def til_x(x):
            # x[b].T [din, s] -> [DC, ST_N, P, 512] contiguous blocks
            t = x[b].T.reshape(DC, P, ST_N, 512).transpose(0, 2, 1, 3)
            return np.ascontiguousarray(t, dtype=np.float16)

        def til_w(w):
            return np.ascontiguousarray(
                w[:, sl].reshape(4, 4, P, DG), dtype=np.float16)

        in_maps.append({
            "qT": til_x(q),
            "kT": til_x(k),
            "vT": til_x(v),
            "wq": til_w(wq),
            "wk": til_w(wk),
            "wv": til_w(wv),
            "wo": np.ascontiguousarray(wo[sl, :].reshape(4, P, DIM),
                                       dtype=np.float16),
            "msk": mask,
            "onesd": ones,
        })
    return in_maps


def kernel(q, k, v, wq, wk, wv, wo, _trace=False, _trace_kwargs=None):
    from concourse.bass_utils import run_bass_kernel_spmd

    q = np.asarray(q, np.float32)
    k = np.asarray(k, np.float32)
    v = np.asarray(v, np.float32)
    nc = _get_nc()
    in_maps = _host_inputs(q, k, v, np.asarray(wq, np.float32),
                           np.asarray(wk, np.float32),
                           np.asarray(wv, np.float32),
                           np.asarray(wo, np.float32))
    kw = {}
    if _trace:
        kw = dict(trace=True, **(_trace_kwargs or {}))
    res = run_bass_kernel_spmd(nc, in_maps, core_ids=list(range(NCORES)), **kw)
    out = np.zeros((B, S, DIM), np.float32)
    for core in range(NCORES):
        b = core // GROUPS
        out[b] += res.results[core]["outp"].astype(np.float32)
    if _trace:
        _cache["last_results"] = res
    return out


# revision 20
# speedup vs baseline: 1.0102x; 1.0102x over previous
"""Bass/Trainium2 kernel for causal multi-head attention.

B=2, S=2048, DIM=2048, H=16 heads, HD=128.
Sharding: 8 cores = (batch b in 0..1) x (head-group g in 0..3).
Each core column-shards wq/wk/wv (4 heads = 512 cols), row-shards wo,
and produces a partial [S, DIM] output; the host sums the 4 partials
per batch (unshard of the row-sharded wo matmul).

Device-side layout trick: the host feeds q/k/v pre-transposed (qT =
q[b].T etc.), so every matmul contraction lands on the partition dim
with zero on-device transposes:
  xqT[hd, s] = wq_chunk.T @ qT_chunk          (proj, transposed out)
  xkT[hd, s] likewise; xv[s, hd] from vT as lhsT
  ST[k, q]   = xkT_chunk.T @ xqT_slice        (scores, transposed)
  softmax over k = partition dim: sum via ones-column matmul
  attnT[hd, q] = xv_chunk.T @ probsT          (PV; output pre-transposed)
  out[s, dout] = attnT_chunk.T @ wo_chunk     (final projection)

Schedule: projections q,k then v st-by-st with attention q-tiles 0 and 1
merged into the later v stages (scope 1); scope 2 runs q-tiles 2..3 with
a 2-deep score->exp->pv software pipeline interleaved with the output
projection, so the PE never waits on the Scalar engine's exp.
"""

import sys

sys.path.insert(0, "/opt/trn_rl_repo")

import numpy as np

B, S, DIM, H = 2, 2048, 2048, 16
HD = 128
NCORES = 8
GROUPS = 4  # head-groups (tensor parallel)
HPG = H // GROUPS  # 4 heads per group
DG = HPG * HD  # 512 = per-group projection width
P = 128
DC = DIM // P  # 16 din chunks
ST_N = S // 512  # 4 s-tiles of 512
SC_N = S // P  # 16 s-chunks of 128
SCALE = 1.0 / np.sqrt(HD)

_cache = {}


def _build(reps=1):
    from contextlib import ExitStack

    import concourse.bacc as bacc
    import concourse.mybir as mybir
    import concourse.tile as tile

    f32 = mybir.dt.float32
    f16 = mybir.dt.float16
    Exp = mybir.ActivationFunctionType.Exp

    nc = bacc.Bacc("TRN2", target_bir_lowering=False, debug=False,
                   num_devices=NCORES)

    # host-tiled layouts: every DMA reads a fully contiguous block
    qT = nc.dram_tensor("qT", [DC, ST_N, P, 512], f16, kind="ExternalInput")
    kT = nc.dram_tensor("kT", [DC, ST_N, P, 512], f16, kind="ExternalInput")
    vT = nc.dram_tensor("vT", [DC, ST_N, P, 512], f16, kind="ExternalInput")
    wq = nc.dram_tensor("wq", [4, 4, P, DG], f16, kind="ExternalInput")
    wk = nc.dram_tensor("wk", [4, 4, P, DG], f16, kind="ExternalInput")
    wv = nc.dram_tensor("wv", [4, 4, P, DG], f16, kind="ExternalInput")
    wo = nc.dram_tensor("wo", [4, P, DIM], f16, kind="ExternalInput")
    msk = nc.dram_tensor("msk", [P, 512], f16, kind="ExternalInput")
    onesd = nc.dram_tensor("onesd", [P, P], f16, kind="ExternalInput")
    outp = nc.dram_tensor("outp", [S, DIM], f16, kind="ExternalOutput")

    with tile.TileContext(nc) as tc, ExitStack() as ctx:
        # Resident for the whole kernel: projected activations + consts.
        resid = ctx.enter_context(tc.tile_pool(name="resid", bufs=1))
        _ = reps  # body below may be repeated for timing builds
        xqT = resid.tile([P, HPG, S], f16, tag="xqT")  # [hd, head, s]
        xkT = resid.tile([P, HPG, S], f16, tag="xkT")
        xv = resid.tile([P, SC_N, DG], f16, tag="xv")  # [s%128, s//128, dout]
        ones = resid.tile([P, P], f16, tag="ones")
        attnT = resid.tile([P, HPG, S], f16, tag="attnT")
        # single lower-triangle mask: tri[p, j] = (j >= p)
        tri = resid.tile([P, 512], f16, tag="tri")
        wo_tiles = [resid.tile([P, DIM], f16, tag=f"wo{c4}",
                               name=f"wo{c4}") for c4 in range(4)]
        # consts go via the Scalar engine's DMA ring so they don't
        # delay the weight/activation loads gating the first matmul
        nc.scalar.dma_start(ones[:], onesd[:])
        nc.scalar.dma_start(tri[:], msk[:])
        # warm the Exp activation table during the projections so the
        # first attention exp doesn't pay the 1.3us ACT_TABLE_LOAD
        actwarm = resid.tile([1, 8], mybir.dt.float32, tag="actwarm")
        nc.vector.memzero(actwarm[:])
        nc.scalar.activation(actwarm[:], actwarm[:],
                             mybir.ActivationFunctionType.Exp)
        # dummy matmuls on a zeroed tile keep the PE busy while the
        # first weight/activation DMAs land, so the p-state governor
        # has the array at full clock when the real work starts
        warm = resid.tile([P, 512], f16, tag="warm")
        nc.vector.memzero(warm[:])

        def merge(*streams):
            """Proportional round-robin merge of (units, weight) streams."""
            streams = [(list(u), w) for u, w in streams if u]
            pos = [0] * len(streams)
            out = []
            while True:
                best, bf = -1, None
                for si, (u, w) in enumerate(streams):
                    if pos[si] >= len(u):
                        continue
                    f = pos[si] / len(u)
                    if bf is None or f < bf:
                        best, bf = si, f
                if best < 0:
                    return out
                out.append(streams[best][0][pos[best]])
                pos[best] += 1
            return out

        def run(units):
            for u in units:
                u()

        for _rep in range(reps):
            if True:
                # ---------- projection unit builders ----------
                def proj_units(name, w_tiles, in_dram, streampool, ppsum):
                    """One tensor's projection as a list of units."""
                    units = []
                    state = {}

                    def w_slice(d, csl):
                        return w_tiles[d // 4][:, d % 4, csl]

                    for st in range(ST_N):
                        def u_alloc(st=st):
                            state[st] = [
                                ppsum.tile([P, 512], f32, tag="pp",
                                           name=f"pp{name}{st}{_i}")
                                for _i in range(4)]
                        for eighth in range(8):
                            def u(st=st, eighth=eighth, alloc=(eighth == 0)):
                                if alloc:
                                    u_alloc(st)
                                psums = state[st]
                                x_sb = streampool.tile([P, 2, 512], f16,
                                                       tag="xs", name="x_sb")
                                src = in_dram[2 * eighth:2 * eighth + 2, st]
                                nc.sync.dma_start(
                                    x_sb[:], src.rearrange("d p j -> p d j"))
                                for i in range(2):
                                    d = 2 * eighth + i
                                    if name == "v":
                                        for j in range(4):
                                            nc.tensor.matmul(
                                                psums[j][:],
                                                x_sb[:, i, j * P:(j + 1) * P],
                                                w_slice(d, slice(None)),
                                                start=(d == 0),
                                                stop=(d == DC - 1))
                                    else:
                                        for c in range(4):
                                            nc.tensor.matmul(
                                                psums[c][:],
                                                w_slice(d,
                                                        slice(c * P,
                                                              (c + 1) * P)),
                                                x_sb[:, i, :],
                                                start=(d == 0),
                                                stop=(d == DC - 1))
                                if eighth == 7:
                                    for i in range(4):
                                        if name == "q":
                                            nc.vector.tensor_copy(
                                                xqT[:, i,
                                                    st * 512:(st + 1) * 512],
                                                psums[i][:])
                                        elif name == "k":
                                            nc.vector.tensor_copy(
                                                xkT[:, i,
                                                    st * 512:(st + 1) * 512],
                                                psums[i][:])
                                        else:
                                            nc.vector.tensor_copy(
                                                xv[:, 4 * st + i, :],
                                                psums[i][:])
                            units.append(u)
                    return units

                def alloc_w(pool, name, w_dram, split_first=False):
                    """Weight tiles + deferred per-chunk DMA closures.
                    split_first halves the first chunk's DMA so the
                    kernel's first matmul (needing only d=0,1) starts
                    ~3us sooner."""
                    tiles = [None] * 4
                    dmas = []
                    for wq4 in range(4):
                        def dma(wq4=wq4):
                            wt = pool.tile([P, 4, DG], f16, tag="w",
                                           name=f"w{name}{wq4}")
                            if wq4 == 0 and split_first:
                                for h in range(2):
                                    nc.sync.dma_start(
                                        wt[:, 2 * h:2 * h + 2],
                                        w_dram[wq4, 2 * h:2 * h + 2]
                                        .rearrange("d p n -> p d n"))
                            else:
                                nc.sync.dma_start(
                                    wt[:],
                                    w_dram[wq4].rearrange("d p n -> p d n"))
                            tiles[wq4] = wt
                        dmas.append(dma)
                    return tiles, dmas

                def weave(units, dmas, slots):
                    """Insert dma closures before the given unit indices."""
                    out = []
                    for i, u in enumerate(units):
                        while dmas and slots and i == slots[0]:
                            out.append(dmas.pop(0))
                            slots.pop(0)
                        out.append(u)
                    return out

                # ---------- attention unit builders ----------
                def a_units2(qt, c, apool, aux, stp, pvp, sump):
                    """3-deep pipelined attention chain (scope 2): the
                    sum/pv matmuls consume probsT from two chunks back,
                    so exp+mask latency never stalls the PE. The softmax
                    denominator matmul uses a full [128,128] ones
                    stationary: same row cost as M=1, but the PE tile
                    config stays 128x128 for every matmul in the kernel
                    (no reconfig stalls) and the denominator arrives
                    pre-broadcast across partitions, so the normalize is
                    a plain DVE mul (no broadcast matmul)."""
                    units = []
                    nkc = 4 * qt + 4
                    kc_order = list(range(4 * qt, nkc)) + list(range(4 * qt))
                    state = {}

                    def geom(ui):
                        kc = kc_order[ui]
                        d = kc - 4 * qt
                        off = max(0, d) * P
                        return kc, d, off, 512 - off

                    for ui in range(nkc + 2):
                        def u(qt=qt, c=c, ui=ui, nkc=nkc):
                            if ui < nkc:
                                kc, d, off, w = geom(ui)
                                qsl = slice(qt * 512 + off, (qt + 1) * 512)
                                st_ps = stp.tile([P, 512], f32, tag="st",
                                                 name="st2_ps")
                                nc.tensor.matmul(
                                    st_ps[:, :w],
                                    xkT[:, c, kc * P:(kc + 1) * P],
                                    xqT[:, c, qsl],
                                    start=True, stop=True)
                                probsT = apool.tile([P, 512], f16,
                                                    tag="probsT",
                                                    name="probsT2")
                                nc.scalar.activation(
                                    probsT[:, :w], st_ps[:, :w], Exp,
                                    scale=SCALE)
                                if d >= 0:
                                    mw = min(P, w)
                                    nc.vector.tensor_mul(
                                        probsT[:, :mw], probsT[:, :mw],
                                        tri[:, :mw])
                                state[ui] = probsT
                            if ui == 0:
                                state["pv"] = pvp.tile([P, 512], f32,
                                                       tag="pv", name="pv2")
                                state["sum"] = sump.tile([P, 512], f32,
                                                         tag="sum",
                                                         name="sum2")
                            elif ui >= 2:
                                kc, d, off, w = geom(ui - 2)
                                probsT = state.pop(ui - 2)
                                nc.tensor.matmul(
                                    state["sum"][:, off:], ones[:, :],
                                    probsT[:, :w],
                                    start=(ui == 2), stop=(ui == nkc + 1))
                                nc.tensor.matmul(
                                    state["pv"][:, off:],
                                    xv[:, kc, c * P:(c + 1) * P],
                                    probsT[:, :w],
                                    start=(ui == 2), stop=(ui == nkc + 1))
                                if ui == nkc + 1:
                                    recip = aux.tile([P, 512], f32,
                                                     tag="recip",
                                                     name="recip2")
                                    nc.vector.reciprocal_approx_fast(
                                        out=recip[:], in_=state["sum"][:])
                                    nc.vector.tensor_mul(
                                        attnT[:, c,
                                              qt * 512:(qt + 1) * 512],
                                        state["pv"][:], recip[:])
                        units.append(u)
                    return units

                def o_units_for_qt(qt, opool, psum_cycle):
                    units = []
                    idx = 0
                    for sc in range(4 * qt, 4 * qt + 4):
                        for dt in range(4):
                            def u(sc=sc, dt=dt, idx=idx):
                                pool, tag = psum_cycle[idx % len(psum_cycle)]
                                o_ps = pool.tile([P, 512], f32, tag=tag,
                                                 name="o_ps")
                                for c in range(HPG):
                                    nc.tensor.matmul(
                                        o_ps[:],
                                        attnT[:, c, sc * P:(sc + 1) * P],
                                        wo_tiles[c][:,
                                                    dt * 512:(dt + 1) * 512],
                                        start=(c == 0), stop=(c == HPG - 1))
                                o_sb = opool.tile([P, 512], f16, tag="o_sb",
                                                  name="o_sb")
                                nc.vector.tensor_copy(o_sb[:], o_ps[:])
                                nc.sync.dma_start(
                                    outp[sc * P:(sc + 1) * P,
                                         dt * 512:(dt + 1) * 512],
                                    o_sb[:])
                            units.append(u)
                            idx += 1
                    return units

                # ================= schedule =================
                apool = ctx.enter_context(tc.tile_pool(name="apool", bufs=8))
                aux = ctx.enter_context(tc.tile_pool(name="aux1", bufs=2))
                # Scope 1: pure projections. The three tensors share one
                # weight pool (wv reuses wq's freed slots) and one stream
                # pool, with weight-chunk DMAs woven between units.
                with (
                    tc.tile_pool(name="wpool", bufs=8) as wpool,
                    tc.tile_pool(name="stream", bufs=6) as stream,
                    tc.tile_pool(name="ppsum", bufs=4, space="PSUM") as ppsum,
                    tc.tile_pool(name="stp0", bufs=2, space="PSUM") as stp0,
                    tc.tile_pool(name="pvp0", bufs=1, space="PSUM") as pvp0,
                    tc.tile_pool(name="sump0", bufs=1,
                                 space="PSUM") as sump0,
                ):
                    for _wi in range(26):
                        wps = ppsum.tile([P, 512], f32, tag="pp",
                                         name="warm_ps")
                        nc.tensor.matmul(wps[:], warm[:, :P], warm[:],
                                         start=True, stop=True)
                    wq_t, wq_d = alloc_w(wpool, "q", wq, split_first=True)
                    uq = proj_units("q", wq_t, qT, stream, ppsum)
                    wk_t, wk_d = alloc_w(wpool, "k", wk)
                    uk = proj_units("k", wk_t, kT, stream, ppsum)
                    wv_t, wv_d = alloc_w(wpool, "v", wv)
                    uv = proj_units("v", wv_t, vT, stream, ppsum)
                    a0 = []
                    for c in range(HPG):
                        a0 += a_units2(0, c, apool, aux, stp0, pvp0,
                                       sump0)
                    # wq chunk i lands just before the unit that needs it;
                    # wk/wv chunks prefetch through the previous phase.
                    run(weave(uq, wq_d + wk_d, [0, 2, 4, 6, 12, 16, 20, 24]))
                    run(weave(uk, wv_d, [12, 16, 20, 24]))
                    # A(0) needs V st=0 evictions -> gate behind uv[:8]
                    run(uv[:8])
                    # prefetch wo for scope 2 while V proj streams
                    for c4 in range(4):
                        nc.sync.dma_start(wo_tiles[c4][:], wo[c4])
                    run(merge((uv[8:], 1), (a0, 1)))

                # Scope 2: attention qt 1..3 (3-deep pipelined chains) +
                # output projection one q-tile behind.
                with (
                    tc.tile_pool(name="opool", bufs=6) as opool,
                    tc.tile_pool(name="stp", bufs=2, space="PSUM") as stp,
                    tc.tile_pool(name="pvp", bufs=3, space="PSUM") as pvp,
                    tc.tile_pool(name="sump", bufs=1, space="PSUM") as sump,
                    tc.tile_pool(name="opsum", bufs=2, space="PSUM") as opsum,
                ):
                    cyc1 = [(opsum, "o")]
                    # tail-only: rotate across all four (now idle) psum tags
                    cyc4 = [(opsum, "o"), (sump, "sum"), (stp, "st"),
                            (pvp, "pv")]
                    for qt in range(1, ST_N):
                        au = []
                        for c in range(HPG):
                            au += a_units2(qt, c, apool, aux, stp, pvp,
                                           sump)
                        ou = o_units_for_qt(qt - 1, opool, cyc1)
                        run(merge((au, 1), (ou, 1)))
                    run(o_units_for_qt(3, opool, cyc4))
    nc.compile()
    return nc


def _get_nc(reps=1):
    key = ("nc", reps)
    if key not in _cache:
        _cache[key] = _build(reps)
    return _cache[key]


def _host_inputs(q, k, v, wq, wk, wv, wo):
    pp = np.arange(P)[:, None]
    jj = np.arange(512)[None, :]
    mask = np.ascontiguousarray((jj >= pp).astype(np.float16))
    ones = np.ones((P, P), np.float16)
    in_maps = []
    for core in range(NCORES):
        b, g = divmod(core, GROUPS)
        sl = slice(g * DG, (g + 1) * DG)
        def til_x(x):
            # x[b].T [din, s] -> [DC, ST_N, P, 512] contiguous blocks
            t = x[b].T.reshape(DC, P, ST_N, 512).transpose(0, 2, 1, 3)
            return np.ascontiguousarray(t, dtype=np.float16)

        def til_w(w):
            return np.ascontiguousarray(
                w[:, sl].reshape(4, 4, P, DG), dtype=np.float16)

        in_maps.append({
            "qT": til_x(q),
            "kT": til_x(k),
            "vT": til_x(v),
            "wq": til_w(wq),
            "wk": til_w(wk),
            "wv": til_w(wv),
            "wo": np.ascontiguousarray(wo[sl, :].reshape(4, P, DIM),
                                       dtype=np.float16),
            "msk": mask,
            "onesd": ones,
        })
    return in_maps


def kernel(q, k, v, wq, wk, wv, wo, _trace=False, _trace_kwargs=None):
    from concourse.bass_utils import run_bass_kernel_spmd

    q = np.asarray(q, np.float32)
    k = np.asarray(k, np.float32)
    v = np.asarray(v, np.float32)
    nc = _get_nc()
    in_maps = _host_inputs(q, k, v, np.asarray(wq, np.float32),
                           np.asarray(wk, np.float32),
                           np.asarray(wv, np.float32),
                           np.asarray(wo, np.float32))
    kw = {}
    if _trace:
        kw = dict(trace=True, **(_trace_kwargs or {}))
    res = run_bass_kernel_spmd(nc, in_maps, core_ids=list(range(NCORES)), **kw)
    out = np.zeros((B, S, DIM), np.float32)
    for core in range(NCORES):
        b = core // GROUPS
        out[b] += res.results[core]["outp"].astype(np.float32)
    if _trace:
        _cache["last_results"] = res
    return out
